# revision 1
# baseline (speedup 1.0000x reference)
"""2D DCT [8,32,256,256] on 8 TRN2 NeuronCores — raw Bass (no Tile).

Math: the reference's FFT-mirror trick is exactly the linear map
    dct1d(x)[k] = (1/L) * sum_m x[m] * cos(pi*k*(m+0.5)/L)
so with A[m,k] = cos(pi*k*(m+0.5)/L)/L the 2D DCT per [256,256] slice is
    out = A^T @ X @ A = (X^T A)^T A
i.e. two chained TensorEngine matmuls with NO transposes:
    V  = matmul(lhsT=X, rhs=A)   # V = X^T A   ([w, j] in PSUM)
    out= matmul(lhsT=V, rhs=A)   # V^T A = A^T X A  ([h', w'] in PSUM)

Sharding: fully data-parallel over batch — core b takes ip[b] (32
independent slices). bf16 staging in a [128, units, 2, 256] host layout
(contiguous per partition; unit 0 is the DCT matrix itself, units 1..32
the slices), f32 PSUM accumulation.

Raw-Bass engine plan (the Tile framework's entry/exit overhead and
per-instruction wait splitting cost several us here; this toolchain's
walrus also rejects >1 sync wait per instruction, which raw streams
with standalone wait_ge instructions avoid):
  SP (sync)  : one HWDGE ring — graduated per-slice in-chunks first
               (each DIRECT2D issue costs ~0.75us of sequencer time,
               which paces the flood so other engines' instruction
               fetches aren't starved), then the out-chunks (issue
               stalls on copy sems), final wait on out completions.
  PE         : warm-up matmuls on garbage SBUF during the DMA head
               (HAM hits K=8/8 about when real data lands), then a
               software-pipelined stream S1(0..3), [S1(s), S2(s-4)],
               S2 tail; one pe_sem inc per 4-matmul stage. Steady
               state measures 109 ns per 128x128x256 bf16 matmul.
  DVE / ACT  : whole-bank PSUM->SBUF evictions (f32->bf16), one per
               stage; BOTH of a slice's evictions go to one engine
               (parity-alternating) so each S2 needs a single wait
               (waits break the LDWEIGHTS pull-ahead, ~170ns refill
               each — merging them got 243/256 matmuls to the 109ns
               floor); streams sorted by pe_sem dependency; ACT issues
               the final slice's out-DMA inline.

Never let two agents touch one PSUM bank concurrently (PE-write +
DVE/ACT-read, or two readers) — it hard-crashes the device
(NRT_EXEC_UNIT_UNRECOVERABLE), which is why evictions are whole-bank
after the full stage.

Measured: 45.1-45.7us HW exec (neuron-profile, core 0) across runs;
~7-9us is fixed runtime preamble, ~28us is the PE streaming floor
(65536 matmul columns at 2.4GHz), rest is warm-up/tail/teardown.
"""

import numpy as np

import concourse.bacc as bacc
import concourse.bass as bass
import concourse.mybir as mybir
from concourse.bass_utils import run_bass_kernel_spmd

N_CORES = 8
C = 32                    # slices per core (channel dim; batch is sharded)
L = 256                   # DCT length
BF16 = mybir.dt.bfloat16
F32 = mybir.dt.float32
NP_BF16 = mybir.dt.np(mybir.dt.bfloat16)

# In-chunks in UNITS of the staged tensor (unit 0 = DCT matrix, issued
# FIRST on the sync ring — the ACT ring can't serve it early because
# walrus prepends the 1.28us InstLoadActFuncSet to the ACT stream;
# unit u = slice u-1), graduated sizes.
IN_CHUNKS = [2, 1, 1, 1, 2, 2, 3, 5, 8, 8]   # chunk 0 = A + slice 0
OUT_CHUNKS = [6, 6, 6, 6, 4, 2, 1]        # slices per sync-ring out-DMA
TAIL_OUT = (31, 32)        # final slice: ACT issues its out-DMA inline
SPLIT_LAST = -1            # disabled
N_WARM = 28               # HAM warm-up matmuls during the DMA head
PS_RV = 4                 # vp PSUM ring depth (banks)
PS_RO = 4                 # op PSUM ring depth (banks)
VS_R = 6                  # vs SBUF ring depth
LOOKAHEAD = PS_RV         # S2(s) issues LOOKAHEAD stages after S1(s)


def _dct_matrix() -> np.ndarray:
    m = np.arange(L, dtype=np.float64)
    k = np.arange(L, dtype=np.float64)
    a = np.cos(np.pi * np.outer(m + 0.5, k) / L) / L
    a = a.astype(np.float32).astype(NP_BF16)
    # pack for SBUF: [p, ki, w] with row ki*128+p on partition p
    return np.ascontiguousarray(a.reshape(2, 128, L).transpose(1, 0, 2))


def _chunk_of_slice(s):
    """Chunk index covering slice s (= unit s+1)."""
    u = s + 1
    c0 = 0
    for ci, n in enumerate(IN_CHUNKS):
        if u < c0 + n:
            return ci
        c0 += n
    raise AssertionError


def _pe_schedule():
    order = []
    for s in range(C):
        order.append(("S1", s))
        if s >= LOOKAHEAD:
            order.append(("S2", s - LOOKAHEAD))
    for s in range(C - LOOKAHEAD, C):
        order.append(("S2", s))
    pe_count = {st: i + 1 for i, st in enumerate(order)}
    return order, pe_count


def _copy_plan(pe_count):
    """vs_copy(s) dep: S1(s); os_copy(s) dep: S2(s). BOTH of slice s's
    evictions go to one engine (dve for even s, act for odd) so that the
    S2(s) vs-ready wait IMPLIES the os(s-LOOKAHEAD) recycle condition:
    same semaphore, and dep(os(s-4)) = S2(s-4) < S1(s) = dep(vs(s)), so
    os(s-4) sorts earlier in the same stream. Halves the PE's wait
    instructions (each wait breaks the LDWEIGHTS pull-ahead, costing a
    ~170ns pipeline refill on the next matmul)."""
    streams = {"dve": [], "act": []}
    for s in range(C):
        eng = "dve" if s % 2 == 0 else "act"
        streams[eng].append((pe_count[("S1", s)], "vs", s))
        streams[eng].append((pe_count[("S2", s)], "os", s))
    pos = {}
    for eng, evs in streams.items():
        evs.sort()
        for i, (dep, kind, s) in enumerate(evs):
            pos[(kind, s)] = (eng, i + 1, dep)
    return streams, pos


def _build(sim: bool = False) -> bass.Bass:
    nc = bacc.Bacc()
    x = nc.declare_dram_parameter("x", [128, C + 1, 2, L], BF16, isOutput=False)
    out = nc.declare_dram_parameter("out", [128, C, 2, L], BF16, isOutput=True)

    order, pe_count = _pe_schedule()
    streams, pos = _copy_plan(pe_count)

    from contextlib import ExitStack

    ctx = ExitStack()
    with ctx:
        warm_sb = ctx.enter_context(nc.sbuf_tensor([128, 128], BF16))
        xs = ctx.enter_context(nc.sbuf_tensor([128, C + 1, 2, L], BF16))
        vs = ctx.enter_context(nc.sbuf_tensor([128, VS_R, 2, L], BF16))
        os_ = ctx.enter_context(nc.sbuf_tensor([128, C, 2, L], BF16))
        vp = ctx.enter_context(nc.psum_tensor([128, PS_RV, 2, L], F32))
        op = ctx.enter_context(nc.psum_tensor([128, PS_RO, 2, L], F32))

        in_sems = [
            ctx.enter_context(nc.semaphore(f"in_sem{i}"))
            for i in range(len(IN_CHUNKS))
        ]
        pe_sem = ctx.enter_context(nc.semaphore("pe_sem"))
        dve_sem = ctx.enter_context(nc.semaphore("dve_sem"))
        act_sem = ctx.enter_context(nc.semaphore("act_sem"))
        out_sem = ctx.enter_context(nc.semaphore("out_sem"))
        warm_sem = ctx.enter_context(nc.semaphore("warm_sem"))
        sem_of = {"dve": dve_sem, "act": act_sem}

        block = ctx.enter_context(nc.Block())

        @block.sync
        def _(eng):
            u0 = 0
            for ci, n in enumerate(IN_CHUNKS):
                eng.dma_start(
                    xs[:, u0 : u0 + n, :, :], x[:, u0 : u0 + n, :, :]
                ).then_inc(in_sems[ci], 16)
                u0 += n
            c0 = 0
            for n in OUT_CHUNKS:
                for eng_name in ("dve", "act"):
                    need = max(
                        (
                            pos[("os", s)][1]
                            for s in range(c0, c0 + n)
                            if pos[("os", s)][0] == eng_name
                        ),
                        default=0,
                    )
                    if need:
                        eng.wait_ge(sem_of[eng_name], need)
                eng.dma_start(
                    out[:, c0 : c0 + n, :, :], os_[:, c0 : c0 + n, :, :]
                ).then_inc(out_sem, 16)
                c0 += n
            eng.wait_ge(out_sem, 16 * (len(OUT_CHUNKS) + 1))

        @block.tensor
        def _(eng):
            if sim:
                # CoreSim rejects reads of uninitialized SBUF; on HW the
                # warm-up matmuls happily consume garbage.
                eng.wait_ge(warm_sem, 1)
            for _ in range(N_WARM):
                # garbage into a vp slot; the first real S1 group's
                # start=True overwrites it
                nc.tensor.matmul(
                    vp[:, 0, 0, 0:128], warm_sb[:], warm_sb[:],
                    start=True, stop=True,
                )
            eng.wait_ge(in_sems[0], 16)   # A (ACT ring)
            seen_chunks = {0}
            for kind, s in order:
                if kind == "S1":
                    ci = _chunk_of_slice(s)
                    if ci not in seen_chunks:
                        seen_chunks.add(ci)
                        eng.wait_ge(in_sems[ci], 16)
                    if s >= PS_RV:
                        # vp ring slot reuse: vs_copy(s-PS_RV) done
                        e, p, _ = pos[("vs", s - PS_RV)]
                        eng.wait_ge(sem_of[e], p)
                    r = s % PS_RV
                    for mi in range(2):
                        for ki in range(2):
                            mm = nc.tensor.matmul(
                                vp[:, r, mi, :],
                                xs[:, s + 1, ki, mi * 128 : (mi + 1) * 128],
                                xs[:, 0, ki, :],
                                start=(ki == 0),
                                stop=(ki == 1),
                            )
                    mm.then_inc(pe_sem, 1)
                else:
                    # one wait covers both S2 preconditions: os(s-PS_RO)
                    # (op slot reuse) sorts AFTER vs(s) (data staged) in
                    # the SAME engine stream, so waiting for it implies
                    # vs(s) is done too
                    if s >= PS_RO:
                        e, p, _ = pos[("os", s - PS_RO)]
                    else:
                        e, p, _ = pos[("vs", s)]
                    eng.wait_ge(sem_of[e], p)
                    r = s % PS_RO
                    for ji in range(2):
                        for wi in range(2):
                            mm = nc.tensor.matmul(
                                op[:, r, ji, :],
                                vs[:, s % VS_R, wi, ji * 128 : (ji + 1) * 128],
                                xs[:, 0, wi, :],
                                start=(wi == 0),
                                stop=(wi == 1),
                            )
                    mm.then_inc(pe_sem, 1)

        def copy_stream(eng_name):
            def body(eng):
                copy = (
                    nc.vector.tensor_copy if eng_name == "dve" else nc.scalar.copy
                )
                if eng_name == "dve" and sim:
                    nc.vector.memset(warm_sb[:], 0.0).then_inc(warm_sem, 1)
                for dep, kind, s in streams[eng_name]:
                    eng.wait_ge(pe_sem, dep)
                    if kind == "vs":
                        copy(vs[:, s % VS_R, :, :], vp[:, s % PS_RV, :, :]).then_inc(
                            sem_of[eng_name], 1
                        )
                    else:
                        copy(os_[:, s, :, :], op[:, s % PS_RO, :, :]).then_inc(
                            sem_of[eng_name], 1
                        )
                if eng_name == "act":
                    # merged tail out-DMA after every tail eviction
                    # (own-engine ones included — the DGE must not read
                    # the staging tile before the writes land)
                    lo, hi = TAIL_OUT
                    for s in range(lo, hi):
                        if s == SPLIT_LAST:
                            eng.wait_ge(dve_sem, pos[("os2", s, "dve")])
                            eng.wait_ge(act_sem, pos[("os2", s, "act")])
                        else:
                            e, p, _ = pos[("os", s)]
                            eng.wait_ge(sem_of[e], p)
                    eng.dma_start(
                        out[:, lo:hi, :, :], os_[:, lo:hi, :, :]
                    ).then_inc(out_sem, 16)
            return body

        block.vector(copy_stream("dve"))
        block.scalar(copy_stream("act"))

    nc.compile()
    return nc


_NC_CACHE: bass.Bass | None = None


def _get_nc() -> bass.Bass:
    global _NC_CACHE
    if _NC_CACHE is None:
        _NC_CACHE = _build()
    return _NC_CACHE


def _make_in_maps(ip: np.ndarray) -> list[dict[str, np.ndarray]]:
    a = _dct_matrix()[:, None, :, :]                   # [128, 1, 2, L]
    in_maps = []
    for b in range(N_CORES):
        xb = ip[b].astype(NP_BF16)                     # [C, 256, 256]
        xb = xb.reshape(C, 2, 128, L).transpose(2, 0, 1, 3)  # [128, C, 2, L]
        xb = np.concatenate([a, xb], axis=1)           # [128, C+1, 2, L]
        in_maps.append({"x": np.ascontiguousarray(xb)})
    return in_maps


def _unpack_out(results: list[dict[str, np.ndarray]]) -> np.ndarray:
    outs = []
    for b in range(N_CORES):
        ob = np.asarray(results[b]["out"])             # [128, C, 2, L] bf16
        ob = ob.transpose(1, 2, 0, 3).reshape(C, 256, 256).astype(np.float32)
        outs.append(ob)
    return np.stack(outs, axis=0)


def run(ip: np.ndarray, trace: bool = False):
    """Run the device kernel; returns (output, BassKernelResults)."""
    ip = np.asarray(ip)
    assert ip.shape == (N_CORES, C, 256, 256), ip.shape
    res = run_bass_kernel_spmd(
        _get_nc(), _make_in_maps(ip), core_ids=list(range(N_CORES)), trace=trace
    )
    return _unpack_out(res.results), res


def kernel(ip: np.ndarray) -> np.ndarray:
    out, _ = run(ip)
    return out



# revision 2
# speedup vs baseline: 1.1092x; 1.1092x over previous
"""2D DCT [8,32,256,256] on 8 TRN2 NeuronCores — raw Bass (no Tile).

Math: with A[m,k] = cos(pi*k*(m+0.5)/L)/L the 2D DCT per [256,256] slice is
    out = A^T @ X @ A
Stage 1 (as before): V = X^T A via 4 matmuls N=256 per slice (lhsT = X
h-chunks, rhs = A), writing one PSUM bank per slice. The host stages the
second half of the w columns REVERSED, so the bank holds
    vp[v, 0:256]   = V[v, j]        (v = 0..127)
    vp[v, 256:512] = V[255-v, j]
Stage 2 uses the DCT-II even/odd symmetry A[255-v, w'] = (-1)^w' A[v, w']:
    out[j, 2t']   = sum_v (V[v,j]+V[255-v,j]) E2[v,t']
    out[j, 2t'+1] = sum_v (V[v,j]-V[255-v,j]) O2[v,t']
so stage 2 is only 2 matmuls of N=512 PER SLICE PAIR (lhsT = E2/O2
stationary, contraction 128) — 37.5% fewer PE streaming columns than the
both-stages-dense baseline (1536 vs 2048 cols/slice).

The butterfly folds (s_w = v0+v1, d_w = v0-v1) cannot read two PSUM
operands (walrus NCC_IBVF027), so the pipeline is:
    PE S1(slice)           -> vp bank (f32)
    ACT cast-pair (FD=1024) vp -> vs_f bf16    (~996ns/pair)
    DVE fold_s/fold_d      vs_f -> vs_sd bf16  (2x mode, ~336ns/pair ea)
    PE S2(pair)            vs_sd -> op banks
    DVE/ACT out-evict      op -> os bf16 (FD=1024, alternating engines)
    sync-ring DMA          os -> DRAM
Measured primitive paces (this container, warm K=8/8 @2.4GHz):
    N=256 matmul 109ns, N=512 matmul 216ns, ACT copy FD/1.2+143ns,
    DVE cast FD/0.96+65ns, DVE bf16 TT 2x FD/1.92+69ns.
Steady-state per pair: PE 1304ns, ACT ~1400ns, DVE ~1340ns.

Implied-wait discipline (each wait costs NX time and breaks the LDWEIGHTS
pull-ahead): PE pair block = [wait act>=cast(p-2)] S1(2p) S1(2p+1)
[wait dve>=fold_d(p-2)] S2(p-2); out-evicts are emitted BEFORE the
fold/cast of the 2-later pair in their engine stream, so the S2/S1 waits
transitively imply op-bank and vp-bank recycling. Never two agents on one
PSUM bank concurrently (hard device crash).
"""

import numpy as np

import concourse.bacc as bacc
import concourse.bass as bass
import concourse.mybir as mybir
from concourse.bass_utils import run_bass_kernel_spmd

N_CORES = 8
C = 32                    # slices per core
P = 16                    # slice pairs per core
L = 256
BF16 = mybir.dt.bfloat16
F32 = mybir.dt.float32
NP_BF16 = mybir.dt.np(mybir.dt.bfloat16)

# staged input units: 0 = A, 1 = [E2|O2], 2+s = slice s
IN_CHUNKS = [3, 1, 1, 1, 2, 2, 3, 5, 8, 8]        # 34 units
OUT_CHUNKS = [3, 3, 3, 3, 2, 1]                   # pairs 0..14 on sync ring
TAIL_PAIR = 15                                    # pair 15 DMA'd from ACT
N_WARM = 26
VPR = 4                   # vp ring (banks) — slice s -> bank s%4
OPR = 4                   # op ring — pair p -> banks 2*(p%2), 2*(p%2)+1
VFR = 8                   # vs_f ring slots — slice s -> slot s%8
SDR = 4                   # vs_sd ring — pair p -> slot p%4
LAG = 2                   # S2(p-LAG) in PE pair block p
# out-evict engine assignment: ~60/40 DVE/ACT (ACT also does all casts)
OUT_ENG = ["dve" if (p % 8) in (0, 2, 4, 5, 7) else "act" for p in range(P)]
OUT_ENG[TAIL_PAIR] = "act"                        # ACT issues tail DMA


def _dct_matrix() -> np.ndarray:
    m = np.arange(L, dtype=np.float64)
    k = np.arange(L, dtype=np.float64)
    a = np.cos(np.pi * np.outer(m + 0.5, k) / L) / L
    return a.astype(np.float32)


def _chunk_of_slice(s):
    u = s + 2
    c0 = 0
    for ci, n in enumerate(IN_CHUNKS):
        if u < c0 + n:
            return ci
        c0 += n
    raise AssertionError


def _schedules():
    """Build per-engine op orders and completion counts.

    pe:  list of ("S1", s) / ("S2", p);  pe_count[op] = sem value when done
    act: list of ("cast", p) / ("out", p); act_count
    dve: list of ("fold", p) / ("out", p); dve_count   (fold = s+d, 2 ops,
         but ONE inc after fold_d — dve_count refers to that inc)
    """
    pe = []
    for p in range(P):
        pe.append(("S1", 2 * p))
        pe.append(("S1", 2 * p + 1))
        if p >= LAG:
            pe.append(("S2", p - LAG))
    for p in range(P - LAG, P):
        pe.append(("S2", p))
    pe_count = {op: i + 1 for i, op in enumerate(pe)}

    act = []
    dve = []
    for p in range(P):
        # out-evicts of pair p-2 go BEFORE this pair's cast/fold so the
        # PE's act>=cast(p)/dve>=fold(p) waits imply op-bank recycling
        if p >= 2:
            q = p - 2
            (act if OUT_ENG[q] == "act" else dve).append(("out", q))
        act.append(("cast", p))
        dve.append(("fold", p))
    for q in (P - 2, P - 1):
        (act if OUT_ENG[q] == "act" else dve).append(("out", q))
    act_count = {op: i + 1 for i, op in enumerate(act)}
    dve_count = {op: i + 1 for i, op in enumerate(dve)}
    return pe, pe_count, act, act_count, dve, dve_count


def _build(sim: bool = False) -> bass.Bass:
    nc = bacc.Bacc()
    x = nc.declare_dram_parameter("x", [128, 2 + C, 512], BF16, isOutput=False)
    out = nc.declare_dram_parameter("out", [128, P, 2, 512], BF16, isOutput=True)

    pe, pe_count, act, act_count, dve, dve_count = _schedules()

    from contextlib import ExitStack

    ctx = ExitStack()
    with ctx:
        warm_sb = ctx.enter_context(nc.sbuf_tensor([128, 128], BF16))
        xs = ctx.enter_context(nc.sbuf_tensor([128, 2 + C, 512], BF16))
        vs_f = ctx.enter_context(nc.sbuf_tensor([128, VFR, 512], BF16))
        vs_sd = ctx.enter_context(nc.sbuf_tensor([128, SDR, 2, 2, 256], BF16))
        os_ = ctx.enter_context(nc.sbuf_tensor([128, P, 2, 512], BF16))
        vp = ctx.enter_context(nc.psum_tensor([128, VPR, 512], F32))
        op = ctx.enter_context(nc.psum_tensor([128, OPR, 512], F32))

        in_sems = [
            ctx.enter_context(nc.semaphore(f"in_sem{i}"))
            for i in range(len(IN_CHUNKS))
        ]
        pe_sem = ctx.enter_context(nc.semaphore("pe_sem"))
        dve_sem = ctx.enter_context(nc.semaphore("dve_sem"))
        act_sem = ctx.enter_context(nc.semaphore("act_sem"))
        out_sem = ctx.enter_context(nc.semaphore("out_sem"))
        warm_sem = ctx.enter_context(nc.semaphore("warm_sem"))
        sem_of = {"dve": dve_sem, "act": act_sem}
        count_of = {"dve": dve_count, "act": act_count}

        block = ctx.enter_context(nc.Block())

        @block.sync
        def _(eng):
            u0 = 0
            for ci, n in enumerate(IN_CHUNKS):
                eng.dma_start(
                    xs[:, u0 : u0 + n, :], x[:, u0 : u0 + n, :]
                ).then_inc(in_sems[ci], 16)
                u0 += n
            c0 = 0
            for n in OUT_CHUNKS:
                for eng_name in ("dve", "act"):
                    need = max(
                        (
                            count_of[eng_name][("out", q)]
                            for q in range(c0, c0 + n)
                            if OUT_ENG[q] == eng_name
                        ),
                        default=0,
                    )
                    if need:
                        eng.wait_ge(sem_of[eng_name], need)
                eng.dma_start(
                    out[:, c0 : c0 + n, :, :], os_[:, c0 : c0 + n, :, :]
                ).then_inc(out_sem, 16)
                c0 += n
            eng.wait_ge(out_sem, 16 * (len(OUT_CHUNKS) + 1))

        @block.tensor
        def _(eng):
            if sim:
                eng.wait_ge(warm_sem, 1)
            for _ in range(N_WARM):
                nc.tensor.matmul(
                    vp[:, 0, 0:128], warm_sb[:], warm_sb[:],
                    start=True, stop=True,
                )
            eng.wait_ge(in_sems[0], 16)
            seen_chunks = {0}
            for kind, i in pe:
                if kind == "S1":
                    s = i
                    ci = _chunk_of_slice(s)
                    if ci not in seen_chunks:
                        seen_chunks.add(ci)
                        eng.wait_ge(in_sems[ci], 16)
                    if s % 2 == 0 and s >= 2 * LAG + 4:
                        # vp banks for this pair freed by cast(s//2 - 2);
                        # earlier pairs implied by stream order
                        eng.wait_ge(act_sem, act_count[("cast", s // 2 - 2)])
                    r = s % VPR
                    for mi in range(2):
                        for ki in range(2):
                            mm = nc.tensor.matmul(
                                vp[:, r, mi * 256 : (mi + 1) * 256],
                                xs[:, 2 + s, ki * 256 + mi * 128 : ki * 256 + (mi + 1) * 128],
                                xs[:, 0, ki * 256 : (ki + 1) * 256],
                                start=(ki == 0),
                                stop=(ki == 1),
                            )
                    mm.then_inc(pe_sem, 1)
                else:
                    q = i
                    # fold_d(q) done implies cast(q), out(q-2) [both engines,
                    # via stream placement] and everything upstream
                    eng.wait_ge(dve_sem, dve_count[("fold", q)])
                    b0 = 2 * (q % 2)
                    mm = nc.tensor.matmul(
                        op[:, b0, :],
                        xs[:, 1, 0:128],
                        vs_sd[:, q % SDR, 0, :, :],
                        start=True, stop=True,
                    )
                    mm = nc.tensor.matmul(
                        op[:, b0 + 1, :],
                        xs[:, 1, 128:256],
                        vs_sd[:, q % SDR, 1, :, :],
                        start=True, stop=True,
                    )
                    mm.then_inc(pe_sem, 1)

        @block.scalar
        def _(eng):
            for kind, p in act:
                if kind == "cast":
                    eng.wait_ge(pe_sem, pe_count[("S1", 2 * p + 1)])
                    cp = nc.scalar.copy(
                        vs_f[:, (2 * p) % VFR : (2 * p) % VFR + 2, :],
                        vp[:, (2 * p) % VPR : (2 * p) % VPR + 2, :],
                    )
                else:
                    eng.wait_ge(pe_sem, pe_count[("S2", p)])
                    cp = nc.scalar.copy(
                        os_[:, p, :, :],
                        op[:, 2 * (p % 2) : 2 * (p % 2) + 2, :],
                    )
                cp.then_inc(act_sem, 1)
            # tail out-DMA for the last pair (its eviction just ran here)
            eng.dma_start(
                out[:, TAIL_PAIR, :, :], os_[:, TAIL_PAIR, :, :]
            ).then_inc(out_sem, 16)

        @block.vector
        def _(eng):
            add = mybir.AluOpType.add
            sub = mybir.AluOpType.subtract
            if sim:
                nc.vector.memset(warm_sb[:], 0.0).then_inc(warm_sem, 1)
            for kind, p in dve:
                if kind == "fold":
                    eng.wait_ge(act_sem, act_count[("cast", p)])
                    f0 = (2 * p) % VFR
                    nc.vector.tensor_tensor(
                        vs_sd[:, p % SDR, 0, :, :],
                        vs_f[:, f0 : f0 + 2, 0:256],
                        vs_f[:, f0 : f0 + 2, 256:512],
                        add,
                    )
                    tt = nc.vector.tensor_tensor(
                        vs_sd[:, p % SDR, 1, :, :],
                        vs_f[:, f0 : f0 + 2, 0:256],
                        vs_f[:, f0 : f0 + 2, 256:512],
                        sub,
                    )
                    tt.then_inc(dve_sem, 1)
                else:
                    eng.wait_ge(pe_sem, pe_count[("S2", p)])
                    nc.vector.tensor_copy(
                        os_[:, p, :, :],
                        op[:, 2 * (p % 2) : 2 * (p % 2) + 2, :],
                    ).then_inc(dve_sem, 1)

    nc.compile()
    return nc


_NC_CACHE: bass.Bass | None = None


def _get_nc() -> bass.Bass:
    global _NC_CACHE
    if _NC_CACHE is None:
        _NC_CACHE = _build()
    return _NC_CACHE


def _make_in_maps(ip: np.ndarray) -> list[dict[str, np.ndarray]]:
    a = _dct_matrix()                                   # [256, 256] f32
    a_bf = a.astype(NP_BF16)
    unit_a = (
        a_bf.reshape(2, 128, 256).transpose(1, 0, 2).reshape(128, 512)
    )                                                   # [p, ki*256+j]
    unit_eo = np.zeros((128, 512), dtype=NP_BF16)
    unit_eo[:, 0:128] = a_bf[0:128, 0::2]               # E2[v, t']
    unit_eo[:, 128:256] = a_bf[0:128, 1::2]             # O2[v, t']
    in_maps = []
    for b in range(N_CORES):
        xb = ip[b].astype(NP_BF16)                      # [C, 256, 256]
        # w-permutation: cols 128.. hold w = 255..128
        xp = np.concatenate([xb[:, :, :128], xb[:, :, 128:][:, :, ::-1]], axis=2)
        # [s, ki, p, mi, c] -> [p, s, ki*256+mi*128+c]
        st = xp.reshape(C, 2, 128, 2, 128).transpose(2, 0, 1, 3, 4).reshape(128, C, 512)
        full = np.concatenate(
            [unit_a[:, None, :], unit_eo[:, None, :], st], axis=1
        )                                               # [128, 34, 512]
        in_maps.append({"x": np.ascontiguousarray(full)})
    return in_maps


def _unpack_out(results: list[dict[str, np.ndarray]]) -> np.ndarray:
    outs = []
    for b in range(N_CORES):
        o = np.asarray(results[b]["out"]).astype(np.float32)  # [128,16,2,512]
        o = o.reshape(128, P, 2, 2, 256)                # [t', pair, eo, sb, j]
        o = o.transpose(1, 3, 4, 0, 2).reshape(C, 256, 256)  # [s, j, w'=2t'+eo]
        outs.append(o)
    return np.stack(outs, axis=0)


def run(ip: np.ndarray, trace: bool = False):
    """Run the device kernel; returns (output, BassKernelResults)."""
    ip = np.asarray(ip)
    assert ip.shape == (N_CORES, C, 256, 256), ip.shape
    res = run_bass_kernel_spmd(
        _get_nc(), _make_in_maps(ip), core_ids=list(range(N_CORES)), trace=trace
    )
    return _unpack_out(res.results), res


def kernel(ip: np.ndarray) -> np.ndarray:
    out, _ = run(ip)
    return out
